# revision 2
# baseline (speedup 1.0000x reference)
"""Trainium2 Bass kernel for a 7-layer binarized CNN (nn_MCNET).

Strategy: pure data parallel over 8 NeuronCores (8 images each). Per core,
each 3x3 VALID conv layer is computed as 9 accumulating matmuls (one per tap),
with the tap shift expressed as a free-dim offset into a flattened
[S*cin, rows*W] activation buffer. S row-bands of the image are stacked along
the partition dim (block-diagonal weights) so small channel counts still fill
the PE array. Layers 1-6 run in bf16 (exact: ternary activations x (+-1)
weights, fp32 PSUM accumulation); layer 0 runs fp32 on the raw input.
Raw Bass with explicit semaphores (standalone wait_ge instructions) because
this walrus build rejects instructions with >1 embedded sync wait.
"""
import sys, os, dataclasses
sys.path.insert(0, '/opt/trn_rl_repo')
import numpy as np

CH = [(3, 4), (4, 8), (8, 16), (16, 32), (32, 64), (64, 32), (32, 2)]
HIN = [256, 127, 125, 123, 121, 119, 117]
HOUT = [h - 2 for h in HIN]          # 254,125,123,121,119,117,115
S = [32, 16, 8, 4, 2, 2, 4]          # bands per layer
B = [8, 8, 16, 32, 64, 64, 29]       # band size (input rows, +2 halo stored)
CR = [2, 4, 4, 4, 4, 4, 4]           # psum-chunk rows (cr*Wout <= 512)
R = [8, 8, 16, 16, 16, 16, 16]       # out rows per band per psum tile
NIMG = 8
OB = 29                               # output band rows (115 = 4 bands of <=29)


def build_program():
    import concourse.bass as bass
    import concourse.mybir as mybir
    dt = mybir.dt
    AF = mybir.ActivationFunctionType

    nc = bass.Bass("TRN2", target_bir_lowering=False)
    x = nc.dram_tensor("x", (NIMG, 3, 256, 256), dt.float32, kind="ExternalInput")
    w0f = nc.dram_tensor("w0f", (96, 9 * 128), dt.float32, kind="ExternalInput")
    WBF_COLS = 9 * (128 * 4 + 64 + 8)  # L1..L6 M sizes: 128,128,128,128,64,8
    wbf = nc.dram_tensor("wbf", (128, WBF_COLS), dt.bfloat16, kind="ExternalInput")
    y = nc.dram_tensor("y", (NIMG, 2 * 115 * 115), dt.float32, kind="ExternalOutput")

    K = [S[l] * CH[l][0] for l in range(7)]   # 96,64,64,64,64,128,128
    M = [S[l] * CH[l][1] for l in range(7)]   # 128,128,128,128,128,64,8
    WOFF = [0]
    for l in range(1, 6):
        WOFF.append(WOFF[-1] + 9 * M[l])

    ctxs = []
    def alloc(cm):
        ctxs.append(cm)
        return cm.__enter__()

    WT0 = alloc(nc.sbuf_tensor("WT0", [128, 9 * 128], dt.float32))
    WTB = alloc(nc.sbuf_tensor("WTB", [128, WBF_COLS], dt.bfloat16))
    A0 = alloc(nc.sbuf_tensor("A0", [128, 2 * 10 * 256], dt.float32))   # 2 slots
    A = [None] * 7
    for l in range(1, 7):
        A[l] = alloc(nc.sbuf_tensor(f"A{l}", [128, (B[l] + 2) * HIN[l]], dt.bfloat16))
    T0 = alloc(nc.sbuf_tensor("T0", [128, 2048], dt.bfloat16))
    T1 = alloc(nc.sbuf_tensor("T1", [128, 4 * 254], dt.bfloat16))
    T3 = alloc(nc.sbuf_tensor("T3", [128, 4 * 127], dt.bfloat16))
    STG = [alloc(nc.sbuf_tensor(f"STG{i}", [128, 16 * 125], dt.bfloat16))
           for i in range(2)]
    OUTB = alloc(nc.sbuf_tensor("OUTB", [128, OB * 115], dt.float32))
    P = [alloc(nc.psum_tensor(f"P{i}", [128, 2048], dt.float32)) for i in range(2)]
    sem = {n: alloc(nc.semaphore(name=n)) for n in
           ['sdma', 'spe', 'sact', 'sdve', 'sgp']}

    # ---------------- plan walk ----------------
    # The walk is deterministic; each engine closure replays it, emitting only
    # its own ops. Counters give exact wait targets.
    def walk(E, me):
        cnt = {'dma': 0, 'pe': 0, 'act': 0, 'dve': 0, 'gp': 0}
        last_wait = {}

        def wait(eng, semn, val):
            if val <= 0:
                return
            k = (eng, semn)
            if last_wait.get(k, -1) >= val:
                return
            last_wait[k] = val
            if eng == me:
                E.wait_ge(sem[semn], val)

        def emit(eng, fn):
            # fn() emits one instruction and returns it (only when eng==me)
            if eng == me:
                return fn()
            return None

        def inc(inst, semn, v):
            if inst is not None:
                inst.then_inc(sem[semn], v)

        # --- init: memsets of activation buffers (NaN poisoning guard) ---
        memset_list = [(A0, 2 * 10 * 256), *[(A[l], (B[l] + 2) * HIN[l]) for l in range(1, 7)]]
        for buf, ncols in memset_list:
            i = emit('gp', lambda buf=buf, ncols=ncols: nc.gpsimd.memset(buf[0:128, 0:ncols], 0.0))
            cnt['gp'] += 1
            inc(i, 'sgp', 1)
        # --- weight DMAs ---
        for (dst, src) in ((WT0, w0f), (WTB, wbf)):
            i = emit('gp', lambda dst=dst, src=src: nc.gpsimd.dma_start(dst[0:src.shape[0], :], src[:]))
            cnt['dma'] += 1
            inc(i, 'sdma', 16)

        slot_free = [None, None]   # (sem_name, val) when psum slot was last freed
        stage_free = [None, None]  # (sem_name, val) when staging slot free
        t3_free = None             # dma count when T3 rebands of prev img done
        a0_free = [None, None]     # sem_pe val when A0 slot free
        l0_tile_pe = [0] * NIMG
        tile_g = 0

        for img in range(NIMG):
            # ---- input DMA (2 dmas: bands 0-30, band 31) ----
            aslot = img % 2
            off = aslot * 2560
            if a0_free[aslot] is not None:
                wait('gp', 'spe', a0_free[aslot])
            src_main = dataclasses.replace(
                x[img], ap=[[2048, 31], [65536, 3], [256, 10], [1, 256]])
            i = emit('gp', lambda src_main=src_main, off=off:
                     nc.gpsimd.dma_start(A0[0:93, off:off + 2560], src_main))
            cnt['dma'] += 1
            inc(i, 'sdma', 16)
            i = emit('gp', lambda img=img, off=off:
                     nc.gpsimd.dma_start(A0[93:96, off:off + 2048], x[img, :, 248:256, :]))
            cnt['dma'] += 1
            inc(i, 'sdma', 16)
            dma_in_done = cnt['dma']

            for l in range(7):
                cin, cout = CH[l]
                W, Wo, s_l, b_l, cr, r_l = HIN[l], HOUT[l], S[l], B[l], CR[l], R[l]
                ntile = -(-b_l // r_l)
                for t in range(ntile):
                    r0 = t * r_l
                    rows = min(r_l, b_l - r0)
                    nch = -(-rows // cr)
                    slot = tile_g % 2
                    PS = P[slot]
                    # ---- PE: waits ----
                    wait('pe', 'sgp', len(memset_list))
                    if l == 0:
                        wait('pe', 'sdma', 16 * dma_in_done)
                        if img == 0 and t == 0:
                            pass
                    else:
                        wait('pe', 'sdma', 16 * prev_ready)
                    if slot_free[slot] is not None:
                        wait('pe', slot_free[slot][0], slot_free[slot][1])
                    # ---- PE: matmuls (taps outer, chunks inner) ----
                    if l == 0:
                        Abuf, aoff = A0, aslot * 2560
                    else:
                        Abuf, aoff = A[l], 0
                    woff = 0 if l == 0 else WOFF[l - 1]
                    for tap in range(9):
                        ki, kj = tap // 3, tap % 3
                        lhsT = (WT0 if l == 0 else WTB)[0:K[l], woff + tap * M[l]: woff + tap * M[l] + M[l]]
                        for c in range(nch):
                            crr = min(cr, rows - c * cr)
                            rbase = aoff + (r0 + c * cr + ki) * W + kj
                            def mk(l=l, c=c, crr=crr, rbase=rbase, tap=tap, W=W, Wo=Wo,
                                   Abuf=Abuf, PS=PS, lhsT=lhsT):
                                rv = Abuf[0:K[l], rbase:rbase + (crr - 1) * W + Wo]
                                rv = dataclasses.replace(
                                    rv, ap=[rv.ap[0], [W, crr], [1, Wo]])
                                ov = PS[0:M[l], c * 512: c * 512 + crr * Wo]
                                return nc.tensor.matmul(ov, lhsT, rv,
                                                        start=(tap == 0), stop=(tap == 8))
                            i = emit('pe', mk)
                            if tap == 8 and c == nch - 1:
                                cnt['pe'] += 1
                                inc(i, 'spe', 1)
                    if l == 0:
                        l0_tile_pe[img] = cnt['pe']
                        a0_free[aslot] = cnt['pe']
                    my_pe = cnt['pe']

                    # ---- evacuation ----
                    if l == 0:
                        # ACT: sign(psum) -> T0 (bf16, psum-chunk layout);
                        # sign commutes with max, so pool after sign.
                        wait('act', 'spe', my_pe)
                        i = emit('act', lambda PS=PS: nc.scalar.activation(
                            T0[0:128, 0:2048], PS[0:128, 0:2048], AF.Sign))
                        cnt['act'] += 1
                        inc(i, 'sact', 1)
                        slot_free[slot] = ('sact', cnt['act'])
                        # DVE: vmax rows then hmax cols (SBUF bf16)
                        wait('dve', 'sact', cnt['act'])
                        if t3_free is not None:
                            wait('dve', 'sdma', 16 * t3_free)
                        def mkv():
                            v = T0[0:128, 0:2048]
                            a = dataclasses.replace(v, ap=[v.ap[0], [512, 4], [1, 254]])
                            b = dataclasses.replace(v, offset=v.offset + 254,
                                                    ap=[v.ap[0], [512, 4], [1, 254]])
                            d = T1[0:128, 0:4 * 254]
                            d = dataclasses.replace(d, ap=[d.ap[0], [254, 4], [1, 254]])
                            return nc.vector.tensor_max(d, a, b)
                        i = emit('dve', mkv)
                        cnt['dve'] += 1
                        inc(i, 'sdve', 1)
                        def mkh():
                            sv = T1[0:128, 0:4 * 254]
                            a = dataclasses.replace(sv, ap=[sv.ap[0], [254, 4], [2, 127]])
                            b = dataclasses.replace(sv, offset=sv.offset + 1,
                                                    ap=[sv.ap[0], [254, 4], [2, 127]])
                            d = T3[0:128, 0:4 * 127]
                            d = dataclasses.replace(d, ap=[d.ap[0], [127, 4], [1, 127]])
                            return nc.vector.tensor_max(d, a, b)
                        i = emit('dve', mkh)
                        cnt['dve'] += 1
                        inc(i, 'sdve', 1)
                        # gp: reband T3 -> A1 via SBUF->SBUF DMA (compute
                        # engines require 32-aligned partition bases; DMA not)
                        wait('gp', 'sdve', cnt['dve'])
                        H1 = 127
                        for s in range(32):
                            g0, g1 = 4 * s, min(4 * s + 4, H1)
                            if g1 <= g0:
                                continue
                            for sp in (s // 2 - 1, s // 2):
                                if sp < 0 or sp >= 16:
                                    continue
                                d0, d1 = 8 * sp, min(8 * sp + 10, H1)
                                a0r, a1r = max(g0, d0), min(g1, d1)
                                if a1r <= a0r:
                                    continue
                                def mkc(s=s, sp=sp, a0r=a0r, a1r=a1r):
                                    sv = T3[4 * s:4 * s + 4,
                                            (a0r - 4 * s) * 127:(a1r - 4 * s) * 127]
                                    dv = A[1][4 * sp:4 * sp + 4,
                                              (a0r - 8 * sp) * 127:(a1r - 8 * sp) * 127]
                                    return nc.gpsimd.dma_start(dv, sv)
                                i = emit('gp', mkc)
                                cnt['dma'] += 1
                                inc(i, 'sdma', 16)
                        t3_free = cnt['dma']
                        prev_ready = cnt['dma']
                    else:
                        # 1) ACT: Sign(psum) -> compact staging (or OUTB for l==6),
                        #    base-0 partition access (PSUM alignment rule)
                        wait('act', 'spe', my_pe)
                        sslot = tile_g % 2
                        if l < 6 and stage_free[sslot] is not None:
                            wait('act', 'sdma', 16 * stage_free[sslot])
                        if l == 6 and img >= 1:
                            wait('act', 'sdma', 16 * outdma_done)
                        Mp = max(32, M[l])
                        nfull = rows // cr
                        rem = rows - nfull * cr
                        DSTC = STG[sslot] if l < 6 else OUTB
                        dst_row0 = 0 if l < 6 else r0
                        if nfull > 0:
                            def mks(PS=PS, Mp=Mp, nfull=nfull, cr=cr, Wo=Wo,
                                    DSTC=DSTC, dst_row0=dst_row0):
                                sv = PS[0:Mp, 0:(nfull - 1) * 512 + cr * Wo]
                                sv = dataclasses.replace(
                                    sv, ap=[sv.ap[0], [512, nfull], [1, cr * Wo]])
                                dv = DSTC[0:Mp, dst_row0 * Wo:(dst_row0 + nfull * cr) * Wo]
                                dv = dataclasses.replace(
                                    dv, ap=[dv.ap[0], [cr * Wo, nfull], [1, cr * Wo]])
                                return nc.scalar.activation(dv, sv, AF.Sign)
                            i = emit('act', mks)
                            cnt['act'] += 1
                            inc(i, 'sact', 1)
                        if rem > 0:
                            def mksr(PS=PS, Mp=Mp, nfull=nfull, rem=rem, Wo=Wo,
                                     DSTC=DSTC, dst_row0=dst_row0, cr=cr):
                                sv = PS[0:Mp, nfull * 512:nfull * 512 + rem * Wo]
                                r0d = dst_row0 + nfull * cr
                                dv = DSTC[0:Mp, r0d * Wo:(r0d + rem) * Wo]
                                return nc.scalar.activation(dv, sv, AF.Sign)
                            i = emit('act', mksr)
                            cnt['act'] += 1
                            inc(i, 'sact', 1)
                        slot_free[slot] = ('sact', cnt['act'])
                        # 2) gp: reband staging -> A[l+1] (SBUF->SBUF DMA)
                        if l < 6:
                            wait('gp', 'sact', cnt['act'])
                            Hn = HOUT[l]
                            Sp, Bp = S[l + 1], B[l + 1]
                            for s in range(s_l):
                                g0 = s * b_l + r0
                                g1 = min(s * b_l + r0 + rows, min((s + 1) * b_l, Hn))
                                if g1 <= g0:
                                    continue
                                for sp in range(Sp):
                                    d0 = sp * Bp
                                    d1 = min(sp * Bp + Bp + 2, Hn)
                                    a0r, a1r = max(g0, d0), min(g1, d1)
                                    if a1r <= a0r:
                                        continue
                                    def mkr(l=l, s=s, sp=sp, a0r=a0r, a1r=a1r,
                                            d0=d0, g0=g0, r0=r0, b_l=b_l, Wo=Wo,
                                            cout=cout, sslot=sslot):
                                        lr0 = a0r - s * b_l - r0
                                        n = a1r - a0r
                                        sv = STG[sslot][s * cout:(s + 1) * cout,
                                                        lr0 * Wo:(lr0 + n) * Wo]
                                        dv = A[l + 1][sp * cout:(sp + 1) * cout,
                                                      (a0r - d0) * Wo:(a1r - d0) * Wo]
                                        return nc.gpsimd.dma_start(dv, sv)
                                    i = emit('gp', mkr)
                                    cnt['dma'] += 1
                                    inc(i, 'sdma', 16)
                            stage_free[sslot] = cnt['dma']
                    tile_g += 1
                # end tiles
                if 1 <= l < 6:
                    prev_ready = cnt['dma']
            # ---- output DMA (4 bands: 29,29,29,28 rows) ----
            wait('gp', 'sact', cnt['act'])
            def mko1(img=img):
                sv = OUTB[0:6, 0:OB * 115]
                dv = y[img, 0:3 * OB * 115]
                dv = dataclasses.replace(
                    dv, ap=[[OB * 115, 3], [13225, 2], [115, OB], [1, 115]],
                    offset=dv.offset)
                sv = dataclasses.replace(sv, ap=[sv.ap[0], [1, OB * 115]])
                return nc.gpsimd.dma_start(dv, sv)
            i = emit('gp', mko1)
            cnt['dma'] += 1
            inc(i, 'sdma', 16)
            def mko2(img=img):
                sv = OUTB[6:8, 0:28 * 115]
                dv = y[img, 0:1]
                dv = dataclasses.replace(
                    dv, offset=dv.offset + 3 * OB * 115,
                    ap=[[13225, 2], [115, 28], [1, 115]])
                return nc.gpsimd.dma_start(dv, sv)
            i = emit('gp', mko2)
            cnt['dma'] += 1
            inc(i, 'sdma', 16)
            outdma_done = cnt['dma']
        return cnt

    with nc.Block() as block:
        @block.tensor
        def _(E):
            walk(E, 'pe')

        @block.scalar
        def _(E):
            walk(E, 'act')

        @block.vector
        def _(E):
            walk(E, 'dve')

        @block.gpsimd
        def _(E):
            walk(E, 'gp')

    for cm in reversed(ctxs):
        cm.__exit__(None, None, None)
    return nc


def pack_weights(ws):
    """ws: list of 7 raw weight arrays (cout, cin, 3, 3). Returns (w0f, wbf)."""
    import ml_dtypes
    sws = [np.sign(w).astype(np.float32) for w in ws]
    K = [S[l] * CH[l][0] for l in range(7)]
    M = [S[l] * CH[l][1] for l in range(7)]
    w0f = np.zeros((96, 9 * 128), np.float32)
    for tap in range(9):
        ki, kj = tap // 3, tap % 3
        blk = sws[0][:, :, ki, kj].T  # (cin, cout)
        for s in range(S[0]):
            w0f[s * 3:s * 3 + 3, tap * 128 + s * 4: tap * 128 + s * 4 + 4] = blk
    WBF_COLS = 9 * (128 * 4 + 64 + 8)
    wbf = np.zeros((128, WBF_COLS), np.float32)
    off = 0
    for l in range(1, 7):
        cin, cout = CH[l]
        for tap in range(9):
            ki, kj = tap // 3, tap % 3
            blk = sws[l][:, :, ki, kj].T
            for s in range(S[l]):
                wbf[s * cin:(s + 1) * cin,
                    off + tap * M[l] + s * cout: off + tap * M[l] + (s + 1) * cout] = blk
        off += 9 * M[l]
    return w0f, wbf.astype(ml_dtypes.bfloat16)


TRACE = False           # test.py sets these; harness leaves them default
TRACE_DIR = None
LAST_RESULT = None


def kernel(**inputs):
    from concourse.bass_utils import run_bass_kernel_spmd
    inp = np.asarray(inputs['inputs'], np.float32)
    ws = [np.asarray(inputs[f'w{i}']) for i in range(7)]
    w0f, wbf = pack_weights(ws)
    nc = build_program()
    in_maps = []
    for c in range(8):
        in_maps.append({'x': np.ascontiguousarray(inp[c * 8:(c + 1) * 8]),
                        'w0f': w0f, 'wbf': wbf})
    kw = {}
    if TRACE:
        kw = dict(trace=True, tmpdir=TRACE_DIR)
    res = run_bass_kernel_spmd(nc, in_maps, core_ids=list(range(8)), **kw)
    global LAST_RESULT
    LAST_RESULT = res
    out = np.concatenate([res.results[c]['y'] for c in range(8)], axis=0)
    return out.astype(np.float32)



# revision 12
# speedup vs baseline: 2.4508x; 2.4508x over previous
"""Trainium2 Bass kernel for a 7-layer binarized CNN (nn_MCNET).

Data parallel over 8 NeuronCores (8 images each). Per core:
- L0 runs fp32 (bit-exact vs the XLA conv: same 9-tap accumulation order),
  32 row-bands stacked block-diagonally; sign+maxpool fused on ACT+DVE.
- L1..L6 run fp8e4 DoubleRow matmuls (exact: activations/weights are +-1/0,
  fp32 PSUM). L1-L4 stack taps (0,kj)/(1,kj) on partition halves (a +W
  shifted copy fills the upper 64 partitions) and pair the remaining tap row
  via the DoubleRow k-pair dim at stride 2W -> 3 half-rate rounds per layer.
  L5/L6 pair taps via a +1-column shifted plane at a large offset -> 5
  half-rate rounds.
- Band sizes form an aligned pyramid (8,16,32,64,64,32 rows) and matmul
  output partitions are permuted so every inter-layer reband is 3-4 large
  affine SBUF->SBUF DMAs.
- Images are processed in interleaved groups of DEPTH so evac/reband chains
  hide under other images' matmuls and the PE stays HAM-warm.

DMA-completion counting is only sound if, when a consumer waits for count N
on a semaphore, no more than N increments can ever have been enqueued at
that point (the queue completes out of order under load). Hence dedicated
per-purpose semaphores: one per (layer, image-slot) activation chain, per
A0 input slot, per OUTB slot, plus an issuing-engine drain-wait before each
shifted-copy DMA that reads what sibling DMAs just wrote.
"""
import sys, os, dataclasses
sys.path.insert(0, '/opt/trn_rl_repo')
import numpy as np

CH = [(3, 4), (4, 8), (8, 16), (16, 32), (32, 64), (64, 32), (32, 2)]
HIN = [256, 127, 125, 123, 121, 119, 117]
HOUT = [h - 2 for h in HIN]          # 254,125,123,121,119,117,115
S_ = [32, 16, 8, 4, 2, 2, 4]         # bands per layer
BO = [8, 8, 16, 32, 64, 64, 32]      # out rows per band
NIMG = 8
DEPTH = 3                            # image pipeline depth
CR = 4                               # psum chunk rows (l>=1)

# tap pairs for L5/L6 (5 DoubleRow rounds; last pairs with zero weights)
PAIRS56 = [((0, 0), (0, 1)), ((0, 2), (1, 0)), ((1, 1), (1, 2)),
           ((2, 0), (2, 1)), ((2, 2), (2, 2))]

# fp8 weight block offsets: L1-L4: 3 rounds x 2 planes x 128; L5: 5x2x64;
# L6: 5x2x16 (M=8 padded to 16: DoubleRow ldweights needs pair stride % 16 == 0)
W8OFF = {1: 0, 2: 768, 3: 1536, 4: 2304, 5: 3072, 6: 3712}
W8_COLS = 3872
MW = {1: 128, 2: 128, 3: 128, 4: 128, 5: 64, 6: 16}  # weight-plane stride


def a_cols(l):
    """plane0 cols of A[l] (elements)."""
    return (BO[l] + 3) * HIN[l]


def build_program():
    import concourse.bass as bass
    import concourse.mybir as mybir
    dt = mybir.dt
    AF = mybir.ActivationFunctionType
    DR = mybir.MatmulPerfMode.DoubleRow

    nc = bass.Bass("TRN2", target_bir_lowering=False)
    x = nc.dram_tensor("x", (NIMG, 3, 256, 256), dt.float32, kind="ExternalInput")
    w0f = nc.dram_tensor("w0f", (96, 9 * 128), dt.float32, kind="ExternalInput")
    w8d = nc.dram_tensor("w8", (128, W8_COLS), dt.float8e4, kind="ExternalInput")
    y = nc.dram_tensor("y", (NIMG, 2 * 115 * 115), dt.float32, kind="ExternalOutput")

    ctxs = []
    def alloc(cm):
        ctxs.append(cm)
        return cm.__enter__()

    WT0 = alloc(nc.sbuf_tensor("WT0", [128, 9 * 128], dt.float32))
    W8 = alloc(nc.sbuf_tensor("W8", [128, W8_COLS], dt.float8e4))
    A0 = alloc(nc.sbuf_tensor("A0", [128, 3 * 2560], dt.float32))   # 3 slots
    A = [None] * 7
    for l in range(1, 7):
        planes = 2 if l >= 5 else 1
        A[l] = alloc(nc.sbuf_tensor(f"A{l}", [128, DEPTH * planes * a_cols(l)],
                                    dt.float8e4))
    T0 = alloc(nc.sbuf_tensor("T0", [128, 2 * 2048], dt.float8e4))   # 2 slots
    T1 = alloc(nc.sbuf_tensor("T1", [128, 2 * 4 * 254], dt.float8e4))
    T3 = alloc(nc.sbuf_tensor("T3", [128, 2 * 4 * 127], dt.float8e4))
    STG = alloc(nc.sbuf_tensor("STG", [128, 4 * 2000], dt.float8e4))  # 4 slots
    OUTB = alloc(nc.sbuf_tensor("OUTB", [128, 2 * 16 * 115], dt.float32))  # 2 slots
    P = [alloc(nc.psum_tensor(f"P{i}", [128, 2048], dt.float32)) for i in range(2)]

    sem_names = ['spe', 'sact', 'sdve', 'sgp', 'swt', 'so0', 'so1',
                 'si0', 'si1', 'si2']
    for l in range(1, 7):
        for p in range(DEPTH):
            sem_names.append(f'r{l}_{p}')
    sem = {n: alloc(nc.semaphore(name=n)) for n in sem_names}

    M = [128, 128, 128, 128, 128, 64, 8]

    # job schedule: groups of DEPTH images, layer-synchronous inside a group
    jobs = []
    for g0 in range(0, NIMG, DEPTH):
        grp = list(range(g0, min(g0 + DEPTH, NIMG)))
        for l in range(7):
            for i in grp:
                jobs.append((i, l))

    def tiles_of(l):
        if l == 0:
            return [(0, 8)]
        out = []
        r0 = 0
        while r0 < BO[l]:
            out.append((r0, min(16, BO[l] - r0)))
            r0 += 16
        return out

    def walk(E, me):
        cnt = {'pe': 0, 'act': 0, 'dve': 0, 'gp': 0}
        csem = {n: 0 for n in sem_names}   # dma-count semaphore values (x16)
        last_wait = {}

        def wait(eng, semn, val):
            if val <= 0:
                return
            k = (eng, semn)
            if last_wait.get(k, -1) >= val:
                return
            last_wait[k] = val
            if eng == me:
                E.wait_ge(sem[semn], val)

        def emit(eng, fn):
            if eng == me:
                return fn()
            return None

        def inc(inst, semn, v):
            if inst is not None:
                inst.then_inc(sem[semn], v)

        def push(fn, sname):
            i = emit('gp', fn)
            csem[sname] += 16
            inc(i, sname, 16)

        # ---- init: memsets (zero-fill is load-bearing: halos/pads read 0) ----
        memset_list = [(A0, 3 * 2560), (T0, 2 * 2048), (T1, 2 * 4 * 254),
                       (T3, 2 * 4 * 127), (STG, 4 * 2000)]
        for l in range(1, 7):
            planes = 2 if l >= 5 else 1
            memset_list.append((A[l], DEPTH * planes * a_cols(l)))
        for buf, ncols in memset_list:
            i = emit('gp', lambda buf=buf, ncols=ncols:
                     nc.gpsimd.memset(buf[0:128, 0:ncols], 0.0))
            cnt['gp'] += 1
            inc(i, 'sgp', 1)
        NMEMSET = len(memset_list)
        # ---- weight DMAs ----
        push(lambda: nc.gpsimd.dma_start(WT0[0:96, :], w0f[:]), 'swt')
        push(lambda: nc.gpsimd.dma_start(W8[0:128, :], w8d[:]), 'swt')

        # ---- state ----
        pe_done = {}          # (img,l) -> spe value after job's last tile
        act_tile = {}         # gtile -> sact value
        ready = {}            # (img,l) -> (sem, val): A[l] chain complete
        a0_ready = {}         # img -> (sem, val)
        dve_vmax = {}         # img -> sdve value after vmax (T0 free)
        dve_done = {}         # img -> sdve after hmax
        t3_free = {}          # img -> (sem, val): img's T3->A1 chain done
        stg_free = {}         # stg slot -> (sem, val)
        outb_free = {}        # outb slot -> (sem, val)
        gtile = 0
        stg_tile = 0
        out_tile = 0

        def emit_input(img):
            slot = img % 3
            off = slot * 2560
            sname = f'si{slot}'
            if img >= 3:
                wait('gp', 'spe', pe_done[(img - 3, 0)])
            src_main = dataclasses.replace(
                x[img], ap=[[2048, 31], [65536, 3], [256, 10], [1, 256]])
            push(lambda src_main=src_main, off=off:
                 nc.gpsimd.dma_start(A0[0:93, off:off + 2560], src_main), sname)
            push(lambda img=img, off=off:
                 nc.gpsimd.dma_start(A0[93:96, off:off + 2048],
                                     x[img, :, 248:256, :]), sname)
            a0_ready[img] = (sname, csem[sname])

        for i in range(min(DEPTH, NIMG)):
            emit_input(i)

        for (img, l) in jobs:
            cin, cout = CH[l]
            W, Wo, bo = HIN[l], HOUT[l], BO[l]
            aslot = img % DEPTH
            job_tiles = tiles_of(l)
            job_stg_slots = []
            first_push_of_job = True
            if l < 6:
                rsem = f'r{l + 1}_{aslot}'

            for t, (r0, rows) in enumerate(job_tiles):
                slot = gtile % 2
                PS = P[slot]
                # ================= PE =================
                wait('pe', 'sgp', NMEMSET)
                wait('pe', 'swt', 32)
                if l == 0:
                    wait('pe', a0_ready[img][0], a0_ready[img][1])
                else:
                    wait('pe', ready[(img, l)][0], ready[(img, l)][1])
                if gtile >= 2:
                    wait('pe', 'sact', act_tile[gtile - 2])

                if l == 0:
                    aoff = (img % 3) * 2560
                    for tap in range(9):
                        ki, kj = divmod(tap, 3)
                        lhsT = WT0[0:96, tap * 128:(tap + 1) * 128]
                        for c in range(4):
                            rbase = aoff + (c * 2 + ki) * 256 + kj
                            def mk(c=c, rbase=rbase, tap=tap, lhsT=lhsT, PS=PS):
                                rv = A0[0:96, rbase:rbase + 256 + 254]
                                rv = dataclasses.replace(
                                    rv, ap=[rv.ap[0], [256, 2], [1, 254]])
                                ov = PS[0:128, c * 512:c * 512 + 508]
                                return nc.tensor.matmul(ov, lhsT, rv,
                                                        start=(tap == 0), stop=(tap == 8))
                            i = emit('pe', mk)
                            if tap == 8 and c == 3:
                                cnt['pe'] += 1
                                inc(i, 'spe', 1)
                else:
                    planes = 2 if l >= 5 else 1
                    abase = aslot * planes * a_cols(l)
                    Lp = a_cols(l)
                    nch = -(-rows // CR)
                    if l <= 4:
                        rounds = [(r, 2 * W) for r in range(3)]
                    else:
                        rounds = []
                        for (ta, tb) in PAIRS56:
                            base = ta[0] * W + ta[1]
                            delta = Lp + (tb[0] - ta[0]) * W + (tb[1] - ta[1]) - 1
                            rounds.append((base, delta))
                    nr = len(rounds)
                    Mw = MW[l]
                    for ri in range(nr):
                        base, delta = rounds[ri]
                        woff = W8OFF[l] + ri * 2 * Mw
                        def mkw(woff=woff, Mw=Mw):
                            lw = W8[0:128, woff:woff + 2 * Mw]
                            return dataclasses.replace(lw, ap=[lw.ap[0], [Mw, 2], [1, Mw]])
                        for c in range(nch):
                            crr = min(CR, rows - c * CR)
                            rb = abase + (r0 + c * CR) * W + base
                            def mk(l=l, c=c, crr=crr, rb=rb, delta=delta, ri=ri,
                                   nr=nr, PS=PS, mkw=mkw, W=W, Wo=Wo, Mw=Mw):
                                lhsT = mkw()
                                span = delta + (crr - 1) * W + Wo
                                rv = A[l][0:128, rb:rb + span]
                                rv = dataclasses.replace(
                                    rv, ap=[rv.ap[0], [delta, 2], [W, crr], [1, Wo]])
                                ov = PS[0:Mw, c * 512:c * 512 + crr * Wo]
                                return nc.tensor.matmul(ov, lhsT, rv,
                                                        start=(ri == 0), stop=(ri == nr - 1),
                                                        perf_mode=DR)
                            i = emit('pe', mk)
                            if ri == nr - 1 and c == nch - 1:
                                cnt['pe'] += 1
                                inc(i, 'spe', 1)
                pe_tile = cnt['pe']
                if t == len(job_tiles) - 1:
                    pe_done[(img, l)] = cnt['pe']

                # ================= ACT (sign evac) =================
                wait('act', 'spe', pe_tile)
                if l == 0:
                    t0off = (img % 2) * 2048
                    if img >= 2:
                        wait('act', 'sdve', dve_vmax[img - 2])
                    i = emit('act', lambda PS=PS, t0off=t0off: nc.scalar.activation(
                        T0[0:128, t0off:t0off + 2048], PS[0:128, 0:2048], AF.Sign))
                    cnt['act'] += 1
                    inc(i, 'sact', 1)
                elif l < 6:
                    sslot = stg_tile % 4
                    soff = sslot * 2000
                    if sslot in stg_free:
                        wait('act', stg_free[sslot][0], stg_free[sslot][1])
                    Mp = max(32, M[l])
                    nfull = rows // CR
                    rem = rows - nfull * CR
                    if nfull > 0:
                        def mks(PS=PS, Mp=Mp, nfull=nfull, Wo=Wo, soff=soff):
                            sv = PS[0:Mp, 0:(nfull - 1) * 512 + CR * Wo]
                            sv = dataclasses.replace(
                                sv, ap=[sv.ap[0], [512, nfull], [1, CR * Wo]])
                            dv = STG[0:Mp, soff:soff + nfull * CR * Wo]
                            dv = dataclasses.replace(
                                dv, ap=[dv.ap[0], [CR * Wo, nfull], [1, CR * Wo]])
                            return nc.scalar.activation(dv, sv, AF.Sign)
                        i = emit('act', mks)
                        cnt['act'] += 1
                        inc(i, 'sact', 1)
                    if rem > 0:
                        def mksr(PS=PS, Mp=Mp, nfull=nfull, rem=rem, Wo=Wo, soff=soff):
                            sv = PS[0:Mp, nfull * 512:nfull * 512 + rem * Wo]
                            dv = STG[0:Mp, soff + nfull * CR * Wo:
                                     soff + (nfull * CR + rem) * Wo]
                            return nc.scalar.activation(dv, sv, AF.Sign)
                        i = emit('act', mksr)
                        cnt['act'] += 1
                        inc(i, 'sact', 1)
                else:
                    oslot = out_tile % 2
                    ooff = oslot * 16 * 115
                    if oslot in outb_free:
                        wait('act', outb_free[oslot][0], outb_free[oslot][1])
                    def mko(PS=PS, rows=rows, ooff=ooff):
                        sv = PS[0:32, 0:(rows // CR - 1) * 512 + CR * 115]
                        sv = dataclasses.replace(
                            sv, ap=[sv.ap[0], [512, rows // CR], [1, CR * 115]])
                        dv = OUTB[0:32, ooff:ooff + rows * 115]
                        dv = dataclasses.replace(
                            dv, ap=[dv.ap[0], [CR * 115, rows // CR], [1, CR * 115]])
                        return nc.scalar.activation(dv, sv, AF.Sign)
                    i = emit('act', mko)
                    cnt['act'] += 1
                    inc(i, 'sact', 1)
                act_tile[gtile] = cnt['act']
                my_act = cnt['act']

                # ================= DVE (L0 pool) =================
                if l == 0:
                    wait('dve', 'sact', my_act)
                    t0off = (img % 2) * 2048
                    t1off = (img % 2) * 4 * 254
                    t3off = (img % 2) * 4 * 127
                    def mkv(t0off=t0off, t1off=t1off):
                        v = T0[0:128, t0off:t0off + 2048]
                        a = dataclasses.replace(v, ap=[v.ap[0], [512, 4], [1, 254]])
                        b = dataclasses.replace(v, offset=v.offset + 254,
                                                ap=[v.ap[0], [512, 4], [1, 254]])
                        d = T1[0:128, t1off:t1off + 4 * 254]
                        d = dataclasses.replace(d, ap=[d.ap[0], [254, 4], [1, 254]])
                        return nc.vector.tensor_max(d, a, b)
                    i = emit('dve', mkv)
                    cnt['dve'] += 1
                    inc(i, 'sdve', 1)
                    dve_vmax[img] = cnt['dve']
                    if img >= 2:
                        wait('dve', t3_free[img - 2][0], t3_free[img - 2][1])
                    def mkh(t1off=t1off, t3off=t3off):
                        sv = T1[0:128, t1off:t1off + 4 * 254]
                        a = dataclasses.replace(sv, ap=[sv.ap[0], [254, 4], [2, 127]])
                        b = dataclasses.replace(sv, offset=sv.offset + 1,
                                                ap=[sv.ap[0], [254, 4], [2, 127]])
                        d = T3[0:128, t3off:t3off + 4 * 127]
                        d = dataclasses.replace(d, ap=[d.ap[0], [127, 4], [1, 127]])
                        return nc.vector.tensor_max(d, a, b)
                    i = emit('dve', mkh)
                    cnt['dve'] += 1
                    inc(i, 'sdve', 1)
                    dve_done[img] = cnt['dve']

                # ================= gp: rebands into A[l+1] =================
                if l < 6:
                    ln = l + 1
                    Wn = HIN[ln]          # = Wo
                    an_planes = 2 if ln >= 5 else 1
                    anbase = aslot * an_planes * a_cols(ln)
                    Lpn = a_cols(ln)
                    if l == 0:
                        wait('gp', 'sdve', dve_done[img])
                    else:
                        wait('gp', 'sact', my_act)
                    if first_push_of_job:
                        first_push_of_job = False
                        if (img - DEPTH, ln) in pe_done:
                            wait('gp', 'spe', pe_done[(img - DEPTH, ln)])
                    if l == 0:
                        t3off = (img % 2) * 4 * 127
                        push(lambda t3off=t3off, anbase=anbase: nc.gpsimd.dma_start(
                            A[1][0:64, anbase:anbase + 508],
                            T3[0:64, t3off:t3off + 508]), rsem)
                        push(lambda t3off=t3off, anbase=anbase: nc.gpsimd.dma_start(
                            A[1][0:64, anbase + 508:anbase + 1016],
                            T3[64:128, t3off:t3off + 508]), rsem)
                        push(lambda t3off=t3off, anbase=anbase: nc.gpsimd.dma_start(
                            A[1][0:60, anbase + 1016:anbase + 1270],
                            T3[4:64, t3off:t3off + 254]), rsem)
                        # upper tap copy reads what the pushes above wrote:
                        # drain this chain first (sound: issued by this engine)
                        wait('gp', rsem, csem[rsem])
                        push(lambda anbase=anbase: nc.gpsimd.dma_start(
                            A[1][64:128, anbase:anbase + 10 * 127],
                            A[1][0:64, anbase + 127:anbase + 11 * 127]), rsem)
                        t3_free[img] = (rsem, csem[rsem])
                        ready[(img, 1)] = (rsem, csem[rsem])
                    elif l <= 3:
                        soff = (stg_tile % 4) * 2000
                        bo_l = BO[l]
                        push(lambda soff=soff, anbase=anbase, r0=r0, rows=rows,
                             Wn=Wn: nc.gpsimd.dma_start(
                            A[ln][0:64, anbase + r0 * Wn:anbase + (r0 + rows) * Wn],
                            STG[0:64, soff:soff + rows * Wn]), rsem)
                        push(lambda soff=soff, anbase=anbase, r0=r0, rows=rows,
                             Wn=Wn, bo_l=bo_l: nc.gpsimd.dma_start(
                            A[ln][0:64, anbase + (bo_l + r0) * Wn:
                                  anbase + (bo_l + r0 + rows) * Wn],
                            STG[64:128, soff:soff + rows * Wn]), rsem)
                        if t == 0:
                            nsb = S_[ln] - 1
                            push(lambda soff=soff, anbase=anbase, Wn=Wn,
                                 cout=cout, nsb=nsb, bo_l=bo_l: nc.gpsimd.dma_start(
                                A[ln][0:nsb * cout,
                                      anbase + 2 * bo_l * Wn:anbase + (2 * bo_l + 2) * Wn],
                                STG[cout:(nsb + 1) * cout, soff:soff + 2 * Wn]), rsem)
                        if t == len(job_tiles) - 1:
                            wait('gp', rsem, csem[rsem])
                            push(lambda anbase=anbase, ln=ln, Wn=Wn: nc.gpsimd.dma_start(
                                A[ln][64:128, anbase:anbase + (BO[ln] + 2) * Wn],
                                A[ln][0:64, anbase + Wn:anbase + (BO[ln] + 3) * Wn]), rsem)
                            ready[(img, ln)] = (rsem, csem[rsem])
                    elif l == 4:
                        soff = (stg_tile % 4) * 2000
                        push(lambda soff=soff, anbase=anbase, r0=r0, rows=rows,
                             Wn=Wn: nc.gpsimd.dma_start(
                            A[5][0:128, anbase + r0 * Wn:anbase + (r0 + rows) * Wn],
                            STG[0:128, soff:soff + rows * Wn]), rsem)
                        if t == 0:
                            push(lambda soff=soff, anbase=anbase, Wn=Wn:
                                 nc.gpsimd.dma_start(
                                A[5][0:64, anbase + 64 * Wn:anbase + 66 * Wn],
                                STG[64:128, soff:soff + 2 * Wn]), rsem)
                        if t == len(job_tiles) - 1:
                            wait('gp', rsem, csem[rsem])
                            push(lambda anbase=anbase, Lpn=Lpn, Wn=Wn:
                                 nc.gpsimd.dma_start(
                                A[5][0:128, anbase + Lpn:anbase + Lpn + 66 * Wn - 1],
                                A[5][0:128, anbase + 1:anbase + 66 * Wn]), rsem)
                            ready[(img, 5)] = (rsem, csem[rsem])
                    else:  # l == 5 -> A6 (1:2 split)
                        soff = (stg_tile % 4) * 2000
                        for s in range(2):
                            g0 = 64 * s + r0
                            nrows = min(rows, 117 - g0)
                            if nrows <= 0:
                                continue
                            sb = g0 // 32
                            ib = g0 % 32
                            push(lambda soff=soff, anbase=anbase, s=s, sb=sb,
                                 ib=ib, nrows=nrows, Wn=Wn, Wo=Wo: nc.gpsimd.dma_start(
                                A[6][sb * 32:(sb + 1) * 32,
                                     anbase + ib * Wn:anbase + (ib + nrows) * Wn],
                                STG[s * 32:(s + 1) * 32, soff:soff + nrows * Wo]), rsem)
                        if r0 == 32:
                            push(lambda soff=soff, anbase=anbase, Wn=Wn, Wo=Wo:
                                 nc.gpsimd.dma_start(
                                A[6][0:32, anbase + 32 * Wn:anbase + 34 * Wn],
                                STG[0:32, soff:soff + 2 * Wo]), rsem)
                            push(lambda soff=soff, anbase=anbase, Wn=Wn, Wo=Wo:
                                 nc.gpsimd.dma_start(
                                A[6][64:96, anbase + 32 * Wn:anbase + 34 * Wn],
                                STG[32:64, soff:soff + 2 * Wo]), rsem)
                        if r0 == 0:
                            push(lambda soff=soff, anbase=anbase, Wn=Wn, Wo=Wo:
                                 nc.gpsimd.dma_start(
                                A[6][32:64, anbase + 32 * Wn:anbase + 34 * Wn],
                                STG[32:64, soff:soff + 2 * Wo]), rsem)
                        if t == len(job_tiles) - 1:
                            wait('gp', rsem, csem[rsem])
                            push(lambda anbase=anbase, Lpn=Lpn, Wn=Wn:
                                 nc.gpsimd.dma_start(
                                A[6][0:128, anbase + Lpn:anbase + Lpn + 35 * Wn - 1],
                                A[6][0:128, anbase + 1:anbase + 35 * Wn]), rsem)
                            ready[(img, 6)] = (rsem, csem[rsem])
                    if l >= 1:
                        job_stg_slots.append(stg_tile % 4)
                        stg_tile += 1
                else:
                    # ================= output DMA =================
                    wait('gp', 'sact', my_act)
                    oslot = out_tile % 2
                    ooff = oslot * 16 * 115
                    osem = f'so{oslot}'
                    def mkoa(img=img, r0=r0, ooff=ooff, rows=rows):
                        dv = y[img, 0:1]
                        dv = dataclasses.replace(
                            dv, offset=dv.offset + r0 * 115,
                            ap=[[3680, 3], [13225, 2], [1, rows * 115]])
                        sv = OUTB[0:6, ooff:ooff + rows * 115]
                        return nc.gpsimd.dma_start(dv, sv)
                    push(mkoa, osem)
                    rows3 = min(rows, 115 - 96 - r0)
                    if rows3 > 0:
                        def mkob(img=img, r0=r0, ooff=ooff, rows3=rows3):
                            dv = y[img, 0:1]
                            dv = dataclasses.replace(
                                dv, offset=dv.offset + 3 * 3680 + r0 * 115,
                                ap=[[13225, 2], [1, rows3 * 115]])
                            sv = OUTB[6:8, ooff:ooff + rows3 * 115]
                            return nc.gpsimd.dma_start(dv, sv)
                        push(mkob, osem)
                    outb_free[oslot] = (osem, csem[osem])
                    out_tile += 1
                gtile += 1
            # job end: STG slots used by this job free once its chain is done
            if 1 <= l < 6:
                for s in job_stg_slots:
                    stg_free[s] = (rsem, csem[rsem])
            # ---- prefetch next group's inputs after this group's L0 phase ----
            if l == 0 and (img + 1) % DEPTH == 0:
                for j in range(img + 1, min(img + 1 + DEPTH, NIMG)):
                    emit_input(j)
        return cnt

    with nc.Block() as block:
        @block.tensor
        def _(E):
            walk(E, 'pe')

        @block.scalar
        def _(E):
            walk(E, 'act')

        @block.vector
        def _(E):
            walk(E, 'dve')

        @block.gpsimd
        def _(E):
            walk(E, 'gp')

    for cm in reversed(ctxs):
        cm.__exit__(None, None, None)
    return nc


def pack_weights(ws):
    """ws: 7 raw arrays (cout, cin, 3, 3). Returns (w0f fp32, w8 fp8e4)."""
    import ml_dtypes
    sws = [np.sign(w).astype(np.float32) for w in ws]
    # L0: rows k = s*3+ci, cols tap*128 + m, m permuted: m = (s%2)*64 + (s//2)*4 + co
    w0f = np.zeros((96, 9 * 128), np.float32)
    for tap in range(9):
        ki, kj = divmod(tap, 3)
        blk = sws[0][:, :, ki, kj].T  # (cin=3, cout=4)
        for s in range(32):
            sp, j1 = s // 2, s % 2
            m0 = j1 * 64 + sp * 4
            w0f[s * 3:(s + 1) * 3, tap * 128 + m0:tap * 128 + m0 + 4] = blk
    w8 = np.zeros((128, W8_COLS), np.float32)
    for l in range(1, 5):
        cin, cout = CH[l]
        S = S_[l]
        loff = W8OFF[l]
        for r in range(3):
            for s in range(S):
                if l <= 3:
                    m0 = (s % 2) * 64 + (s // 2) * cout
                else:
                    m0 = s * cout
                for half in range(2):
                    blk = sws[l][:, :, half, r].T
                    k0 = half * 64 + s * cin
                    w8[k0:k0 + cin, loff + r * 256 + m0:loff + r * 256 + m0 + cout] = blk
                blk2 = sws[l][:, :, 2, r].T
                w8[s * cin:s * cin + cin,
                   loff + r * 256 + 128 + m0:loff + r * 256 + 128 + m0 + cout] = blk2
    for l in (5, 6):
        cin, cout = CH[l]
        S = S_[l]
        Ml = MW[l]
        loff = W8OFF[l]
        for p, (ta, tb) in enumerate(PAIRS56):
            for s in range(S):
                m0 = s * cout
                blka = sws[l][:, :, ta[0], ta[1]].T
                w8[s * cin:(s + 1) * cin, loff + p * 2 * Ml + m0:
                   loff + p * 2 * Ml + m0 + cout] = blka
                if p < 4:
                    blkb = sws[l][:, :, tb[0], tb[1]].T
                    w8[s * cin:(s + 1) * cin, loff + p * 2 * Ml + Ml + m0:
                       loff + p * 2 * Ml + Ml + m0 + cout] = blkb
    return w0f, w8.astype(ml_dtypes.float8_e4m3)


TRACE = False           # test.py sets these; harness leaves them default
TRACE_DIR = None
LAST_RESULT = None


def kernel(**inputs):
    from concourse.bass_utils import run_bass_kernel_spmd
    inp = np.asarray(inputs['inputs'], np.float32)
    ws = [np.asarray(inputs[f'w{i}']) for i in range(7)]
    w0f, w8 = pack_weights(ws)
    nc = build_program()
    in_maps = []
    for c in range(8):
        in_maps.append({'x': np.ascontiguousarray(inp[c * 8:(c + 1) * 8]),
                        'w0f': w0f, 'w8': w8})
    kw = {}
    if TRACE:
        kw = dict(trace=True, tmpdir=TRACE_DIR)
    res = run_bass_kernel_spmd(nc, in_maps, core_ids=list(range(8)), **kw)
    global LAST_RESULT
    LAST_RESULT = res
    out = np.concatenate([res.results[c]['y'] for c in range(8)], axis=0)
    return out.astype(np.float32)


# revision 33
# speedup vs baseline: 3.7458x; 1.5284x over previous
"""Trainium2 Bass kernel for a 7-layer binarized CNN (nn_MCNET).

Data parallel over 8 NeuronCores (8 images each). Per core:
- L0 runs fp32 (bit-exact vs the XLA conv: same 9-tap accumulation order),
  32 row-bands stacked block-diagonally; sign+maxpool fused on ACT+DVE.
- L1..L6 run fp8e4 DoubleRow matmuls (exact: activations/weights are +-1/0,
  fp32 PSUM). L1-L4 stack taps (0,kj)/(1,kj) on partition halves (a +W
  shifted copy fills the upper 64 partitions) and pair the remaining tap row
  via the DoubleRow k-pair dim at stride 2W -> 3 half-rate rounds per layer.
  L5/L6 pair taps via a +1-column shifted plane at a large offset -> 5
  half-rate rounds.
- Band sizes form an aligned pyramid (8,16,32,64,64,32 rows) and matmul
  output partitions are permuted so every inter-layer reband is 3-4 large
  affine SBUF->SBUF DMAs.
- Images are processed in interleaved groups of DEPTH so evac/reband chains
  hide under other images' matmuls and the PE stays HAM-warm.

DMA-completion counting is only sound if, when a consumer waits for count N
on a semaphore, no more than N increments can ever have been enqueued at
that point (the queue completes out of order under load). Hence dedicated
per-purpose semaphores: one per (layer, image-slot) activation chain, per
A0 input slot, per OUTB slot, plus an issuing-engine drain-wait before each
shifted-copy DMA that reads what sibling DMAs just wrote.
"""
import sys, os, dataclasses
sys.path.insert(0, '/opt/trn_rl_repo')
import numpy as np

CH = [(3, 4), (4, 8), (8, 16), (16, 32), (32, 64), (64, 32), (32, 2)]
HIN = [256, 127, 125, 123, 121, 119, 117]
HOUT = [h - 2 for h in HIN]          # 254,125,123,121,119,117,115
S_ = [32, 16, 8, 4, 2, 2, 4]         # bands per layer
BO = [8, 8, 16, 32, 64, 64, 32]      # out rows per band
NIMG = 8
DEPTH = 3                            # image pipeline depth
CR = 4                               # psum chunk rows (l>=1)

# tap pairs for L5/L6 (5 DoubleRow rounds; last pairs with zero weights)
PAIRS56 = [((0, 0), (0, 1)), ((0, 2), (1, 0)), ((1, 1), (1, 2)),
           ((2, 0), (2, 1)), ((2, 2), (2, 2))]

# fp8 weight block offsets: L1-L4: 3 rounds x 2 planes x 128; L5: 5x2x64;
# L6: 5x2x16 (M=8 padded to 16: DoubleRow ldweights needs pair stride % 16 == 0)
W8OFF = {1: 0, 2: 768, 3: 1536, 4: 2304, 5: 3072, 6: 3712}
W8_COLS = 3872
MW = {1: 128, 2: 128, 3: 128, 4: 128, 5: 64, 6: 16}  # weight-plane stride


def a_cols(l):
    """plane0 cols of A[l] (elements)."""
    return (BO[l] + 3) * HIN[l]


def build_program():
    import concourse.bass as bass
    import concourse.mybir as mybir
    dt = mybir.dt
    AF = mybir.ActivationFunctionType
    DR = mybir.MatmulPerfMode.DoubleRow

    nc = bass.Bass("TRN2", target_bir_lowering=False)
    x = nc.dram_tensor("x", (NIMG, 3, 256, 256), dt.float32, kind="ExternalInput")
    w0f = nc.dram_tensor("w0f", (96, 9 * 128), dt.float32, kind="ExternalInput")
    w8d = nc.dram_tensor("w8", (128, W8_COLS), dt.float8e4, kind="ExternalInput")
    y = nc.dram_tensor("y", (NIMG, 2 * 115 * 115), dt.float32, kind="ExternalOutput")

    ctxs = []
    def alloc(cm):
        ctxs.append(cm)
        return cm.__enter__()

    WT0 = alloc(nc.sbuf_tensor("WT0", [128, 9 * 128], dt.float32))
    W8 = alloc(nc.sbuf_tensor("W8", [128, W8_COLS], dt.float8e4))
    A0 = alloc(nc.sbuf_tensor("A0", [128, 3 * 2560], dt.float32))   # 3 slots
    A = [None] * 7
    for l in range(1, 7):
        planes = 2 if l >= 5 else 1
        A[l] = alloc(nc.sbuf_tensor(f"A{l}", [128, DEPTH * planes * a_cols(l)],
                                    dt.float8e4))
    T0 = alloc(nc.sbuf_tensor("T0", [128, 2 * 2048], dt.float8e4))   # 2 slots
    T1 = alloc(nc.sbuf_tensor("T1", [128, 2 * 4 * 254], dt.float8e4))
    T3 = alloc(nc.sbuf_tensor("T3", [128, 2 * 4 * 127], dt.float8e4))
    STG = alloc(nc.sbuf_tensor("STG", [128, 6 * 2000], dt.float8e4))  # 6 slots
    OUTB = alloc(nc.sbuf_tensor("OUTB", [128, 3 * 16 * 115], dt.float32))  # 3 slots
    P = [alloc(nc.psum_tensor(f"P{i}", [128, 2048], dt.float32)) for i in range(2)]

    sem_names = ['spe', 'sact', 'sdve', 'sgp', 'swt', 'so0', 'so1', 'so2',
                 'si0', 'si1', 'si2']
    for l in range(1, 7):
        for p in range(DEPTH):
            sem_names.append(f'r{l}_{p}')
    sem = {n: alloc(nc.semaphore(name=n)) for n in sem_names}

    M = [128, 128, 128, 128, 128, 64, 8]

    # job schedule: groups of DEPTH images, layer-synchronous inside a group
    jobs = []
    for g0 in range(0, NIMG, DEPTH):
        grp = list(range(g0, min(g0 + DEPTH, NIMG)))
        for l in range(7):
            for i in grp:
                jobs.append((i, l))

    def tiles_of(l):
        if l == 0:
            return [(0, 8)]
        out = []
        r0 = 0
        while r0 < BO[l]:
            out.append((r0, min(16, BO[l] - r0)))
            r0 += 16
        return out

    def walk(E, me):
        cnt = {'pe': 0, 'act': 0, 'dve': 0, 'gp': 0}
        csem = {n: 0 for n in sem_names}   # dma-count semaphore values (x16)
        last_wait = {}

        def wait(eng, semn, val):
            if val <= 0:
                return
            k = (eng, semn)
            if last_wait.get(k, -1) >= val:
                return
            last_wait[k] = val
            if eng == me:
                E.wait_ge(sem[semn], val)

        def emit(eng, fn):
            if eng == me:
                return fn()
            return None

        def inc(inst, semn, v):
            if inst is not None:
                inst.then_inc(sem[semn], v)

        def push(fn, sname, eng='gp'):
            i = emit(eng, fn)
            csem[sname] += 16
            inc(i, sname, 16)

        # ---- init: exact pad memsets (zeros are load-bearing only in
        # halo tails / pad rows; everything else is written before read) ----
        pads = []
        for d in range(3):
            pads.append((A0, 64, 96, d * 2560 + 2048, d * 2560 + 2560))
        for d in range(DEPTH):
            b1 = d * a_cols(1)
            pads += [(A[1], 32, 64, b1 + 8 * 127, b1 + 10 * 127),
                     (A[1], 64, 128, b1 + 9 * 127, b1 + 11 * 127),
                     (A[1], 96, 128, b1 + 7 * 127, b1 + 10 * 127)]
            b2 = d * a_cols(2)
            pads += [(A[2], 32, 64, b2 + 16 * 125, b2 + 18 * 125),
                     (A[2], 64, 128, b2 + 17 * 125, b2 + 19 * 125),
                     (A[2], 96, 128, b2 + 15 * 125, b2 + 18 * 125)]
            b3 = d * a_cols(3)
            pads += [(A[3], 32, 64, b3 + 32 * 123, b3 + 34 * 123),
                     (A[3], 64, 128, b3 + 33 * 123, b3 + 35 * 123),
                     (A[3], 96, 128, b3 + 31 * 123, b3 + 34 * 123)]
            b4 = d * a_cols(4)
            pads += [(A[4], 32, 64, b4 + 64 * 121, b4 + 66 * 121),
                     (A[4], 64, 128, b4 + 65 * 121, b4 + 67 * 121),
                     (A[4], 96, 128, b4 + 63 * 121, b4 + 66 * 121)]
            b5 = d * 2 * a_cols(5)
            pads += [(A[5], 64, 128, b5 + 64 * 119, b5 + 66 * 119)]
            b6 = d * 2 * a_cols(6)
            pads += [(A[6], 96, 128, b6 + 21 * 117, b6 + 34 * 117)]
        NMEMSET = 3  # PE L0 only needs the A0 pads (first 3); rest are
        # ordered before all reband pushes in the gp stream (FIFO)
        for (buf, p0, p1, c0, c1) in pads[:3]:
            i = emit('gp', lambda buf=buf, p0=p0, p1=p1, c0=c0, c1=c1:
                     nc.gpsimd.memset(buf[p0:p1, c0:c1], 0.0))
            cnt['gp'] += 1
            inc(i, 'sgp', 1)
        # ---- weight DMAs ----
        push(lambda: nc.gpsimd.dma_start(WT0[0:96, :], w0f[:]), 'swt')
        push(lambda: nc.gpsimd.dma_start(W8[0:128, :], w8d[:]), 'swt')
        _PADS_REST = pads[3:]

        # ---- state ----
        pe_done = {}          # (img,l) -> spe value after job's last tile
        act_tile = {}         # gtile -> sact value
        ready = {}            # (img,l) -> (sem, val): A[l] chain complete
        a0_ready = {}         # img -> (sem, val)
        dve_vmax = {}         # img -> sdve value after vmax (T0 free)
        dve_done = {}         # img -> sdve after hmax
        t3_free = {}          # img -> (sem, val): img's T3->A1 chain done
        stg_free = {}         # stg slot -> (sem, val)
        outb_free = {}        # outb slot -> (sem, val)
        gtile = 0
        stg_tile = 0
        out_tile = 0

        def emit_input(img):
            slot = img % 3
            off = slot * 2560
            sname = f'si{slot}'
            if img >= 3:
                wait('gp', 'spe', pe_done[(img - 3, 0)])
            for k in range(4):
                b0, b1 = 8 * k, min(8 * k + 8, 31)
                qe = 'sy'
                QDE = nc.sync
                src = dataclasses.replace(
                    x[img], offset=x[img].offset + b0 * 2048,
                    ap=[[2048, b1 - b0], [65536, 3], [256, 10], [1, 256]])
                push(lambda QDE=QDE, src=src, off=off, b0=b0, b1=b1:
                     QDE.dma_start(A0[3 * b0:3 * b1, off:off + 2560], src),
                     sname, qe)
            push(lambda img=img, off=off:
                 nc.sync.dma_start(A0[93:96, off:off + 2048],
                                   x[img, :, 248:256, :]), sname, 'sy')
            a0_ready[img] = (sname, csem[sname])

        for i in range(min(DEPTH, NIMG)):
            emit_input(i)
        for (buf, p0, p1, c0, c1) in _PADS_REST:
            i = emit('gp', lambda buf=buf, p0=p0, p1=p1, c0=c0, c1=c1:
                     nc.gpsimd.memset(buf[p0:p1, c0:c1], 0.0))
            cnt['gp'] += 1
            inc(i, 'sgp', 1)

        for (img, l) in jobs:
            cin, cout = CH[l]
            W, Wo, bo = HIN[l], HOUT[l], BO[l]
            aslot = img % DEPTH
            job_tiles = tiles_of(l)
            job_stg_slots = []
            first_push_of_job = True
            if l < 6:
                rsem = f'r{l + 1}_{aslot}'

            for t, (r0, rows) in enumerate(job_tiles):
                slot = gtile % 2
                PS = P[slot]
                # ================= PE =================
                wait('pe', 'sgp', NMEMSET)
                wait('pe', 'swt', 32)
                if l == 0:
                    wait('pe', a0_ready[img][0], a0_ready[img][1])
                else:
                    wait('pe', ready[(img, l)][0], ready[(img, l)][1])
                if gtile >= 2:
                    wait('pe', 'sact', act_tile[gtile - 2])

                if l == 0:
                    aoff = (img % 3) * 2560
                    for tap in range(9):
                        ki, kj = divmod(tap, 3)
                        for c in range(4):
                            rbase = aoff + (c * 2 + ki) * 256 + kj
                            def mk(c=c, rbase=rbase, tap=tap, PS=PS):
                                lhsT = WT0[0:96, tap * 128:(tap + 1) * 128]
                                rv = A0[0:96, rbase:rbase + 256 + 254]
                                rv = dataclasses.replace(
                                    rv, ap=[rv.ap[0], [256, 2], [1, 254]])
                                ov = PS[0:128, c * 512:c * 512 + 508]
                                return nc.tensor.matmul(ov, lhsT, rv,
                                                        start=(tap == 0), stop=(tap == 8))
                            i = emit('pe', mk)
                            if tap == 8 and c == 3:
                                cnt['pe'] += 1
                                inc(i, 'spe', 1)
                else:
                    planes = 2 if l >= 5 else 1
                    abase = aslot * planes * a_cols(l)
                    Lp = a_cols(l)
                    nch = -(-rows // CR)
                    if l <= 4:
                        rounds = [(r, 2 * W) for r in range(3)]
                    else:
                        rounds = []
                        for (ta, tb) in PAIRS56:
                            base = ta[0] * W + ta[1]
                            delta = Lp + (tb[0] - ta[0]) * W + (tb[1] - ta[1]) - 1
                            rounds.append((base, delta))
                    nr = len(rounds)
                    Mw = MW[l]
                    for ri in range(nr):
                        base, delta = rounds[ri]
                        woff = W8OFF[l] + ri * 2 * Mw
                        def mkw(woff=woff, Mw=Mw):
                            lw = W8[0:128, woff:woff + 2 * Mw]
                            return dataclasses.replace(lw, ap=[lw.ap[0], [Mw, 2], [1, Mw]])
                        for c in range(nch):
                            crr = min(CR, rows - c * CR)
                            rb = abase + (r0 + c * CR) * W + base
                            def mk(l=l, c=c, crr=crr, rb=rb, delta=delta, ri=ri,
                                   nr=nr, PS=PS, mkw=mkw, W=W, Wo=Wo, Mw=Mw):
                                lhsT = mkw()
                                span = delta + (crr - 1) * W + Wo
                                rv = A[l][0:128, rb:rb + span]
                                rv = dataclasses.replace(
                                    rv, ap=[rv.ap[0], [delta, 2], [W, crr], [1, Wo]])
                                ov = PS[0:Mw, c * 512:c * 512 + crr * Wo]
                                return nc.tensor.matmul(ov, lhsT, rv,
                                                        start=(ri == 0), stop=(ri == nr - 1),
                                                        perf_mode=DR)
                            i = emit('pe', mk)
                            if ri == nr - 1 and c == nch - 1:
                                cnt['pe'] += 1
                                inc(i, 'spe', 1)
                pe_tile = cnt['pe']
                if t == len(job_tiles) - 1:
                    pe_done[(img, l)] = cnt['pe']

                # ================= ACT (sign evac) =================
                wait('act', 'spe', pe_tile)
                if l == 0:
                    t0off = (img % 2) * 2048
                    if img >= 2:
                        wait('act', 'sdve', dve_vmax[img - 2])
                    i = emit('act', lambda PS=PS, t0off=t0off: nc.scalar.activation(
                        T0[0:128, t0off:t0off + 2048], PS[0:128, 0:2048], AF.Sign))
                    cnt['act'] += 1
                    inc(i, 'sact', 1)
                elif l < 6:
                    sslot = stg_tile % 6
                    soff = sslot * 2000
                    if sslot in stg_free:
                        wait('act', stg_free[sslot][0], stg_free[sslot][1])
                    Mp = max(32, M[l])
                    nfull = rows // CR
                    rem = rows - nfull * CR
                    if nfull > 0:
                        def mks(PS=PS, Mp=Mp, nfull=nfull, Wo=Wo, soff=soff):
                            sv = PS[0:Mp, 0:(nfull - 1) * 512 + CR * Wo]
                            sv = dataclasses.replace(
                                sv, ap=[sv.ap[0], [512, nfull], [1, CR * Wo]])
                            dv = STG[0:Mp, soff:soff + nfull * CR * Wo]
                            dv = dataclasses.replace(
                                dv, ap=[dv.ap[0], [CR * Wo, nfull], [1, CR * Wo]])
                            return nc.scalar.activation(dv, sv, AF.Sign)
                        i = emit('act', mks)
                        cnt['act'] += 1
                        inc(i, 'sact', 1)
                    if rem > 0:
                        def mksr(PS=PS, Mp=Mp, nfull=nfull, rem=rem, Wo=Wo, soff=soff):
                            sv = PS[0:Mp, nfull * 512:nfull * 512 + rem * Wo]
                            dv = STG[0:Mp, soff + nfull * CR * Wo:
                                     soff + (nfull * CR + rem) * Wo]
                            return nc.scalar.activation(dv, sv, AF.Sign)
                        i = emit('act', mksr)
                        cnt['act'] += 1
                        inc(i, 'sact', 1)
                else:
                    oslot = out_tile % 3
                    ooff = oslot * 16 * 115
                    if oslot in outb_free:
                        wait('act', outb_free[oslot][0], outb_free[oslot][1])
                    def mko(PS=PS, rows=rows, ooff=ooff):
                        sv = PS[0:32, 0:(rows // CR - 1) * 512 + CR * 115]
                        sv = dataclasses.replace(
                            sv, ap=[sv.ap[0], [512, rows // CR], [1, CR * 115]])
                        dv = OUTB[0:32, ooff:ooff + rows * 115]
                        dv = dataclasses.replace(
                            dv, ap=[dv.ap[0], [CR * 115, rows // CR], [1, CR * 115]])
                        return nc.scalar.activation(dv, sv, AF.Sign)
                    i = emit('act', mko)
                    cnt['act'] += 1
                    inc(i, 'sact', 1)
                act_tile[gtile] = cnt['act']
                my_act = cnt['act']

                # ================= DVE (L0 pool) =================
                if l == 0:
                    wait('dve', 'sact', my_act)
                    t0off = (img % 2) * 2048
                    t1off = (img % 2) * 4 * 254
                    t3off = (img % 2) * 4 * 127
                    def mkv(t0off=t0off, t1off=t1off):
                        v = T0[0:128, t0off:t0off + 2048]
                        a = dataclasses.replace(v, ap=[v.ap[0], [512, 4], [1, 254]])
                        b = dataclasses.replace(v, offset=v.offset + 254,
                                                ap=[v.ap[0], [512, 4], [1, 254]])
                        d = T1[0:128, t1off:t1off + 4 * 254]
                        d = dataclasses.replace(d, ap=[d.ap[0], [254, 4], [1, 254]])
                        return nc.vector.tensor_max(d, a, b)
                    i = emit('dve', mkv)
                    cnt['dve'] += 1
                    inc(i, 'sdve', 1)
                    dve_vmax[img] = cnt['dve']
                    if img >= 2:
                        wait('dve', t3_free[img - 2][0], t3_free[img - 2][1])
                    def mkh(t1off=t1off, t3off=t3off):
                        sv = T1[0:128, t1off:t1off + 4 * 254]
                        a = dataclasses.replace(sv, ap=[sv.ap[0], [254, 4], [2, 127]])
                        b = dataclasses.replace(sv, offset=sv.offset + 1,
                                                ap=[sv.ap[0], [254, 4], [2, 127]])
                        d = T3[0:128, t3off:t3off + 4 * 127]
                        d = dataclasses.replace(d, ap=[d.ap[0], [127, 4], [1, 127]])
                        return nc.vector.tensor_max(d, a, b)
                    i = emit('dve', mkh)
                    cnt['dve'] += 1
                    inc(i, 'sdve', 1)
                    dve_done[img] = cnt['dve']

                # ============ rebands into A[l+1] (queue by img parity) ============
                if l < 6:
                    eng = 'gp' if img % 2 == 0 else 'sy'
                    DE = nc.gpsimd if img % 2 == 0 else nc.sync
                    ln = l + 1
                    Wn = HIN[ln]          # = Wo
                    an_planes = 2 if ln >= 5 else 1
                    anbase = aslot * an_planes * a_cols(ln)
                    Lpn = a_cols(ln)
                    if l == 0:
                        wait(eng, 'sdve', dve_done[img])
                    else:
                        wait(eng, 'sact', my_act)
                    if first_push_of_job:
                        first_push_of_job = False
                        if (img - DEPTH, ln) in pe_done:
                            wait(eng, 'spe', pe_done[(img - DEPTH, ln)])
                    if l == 0:
                        t3off = (img % 2) * 4 * 127
                        push(lambda DE=DE, t3off=t3off, anbase=anbase: DE.dma_start(
                            A[1][0:64, anbase:anbase + 508],
                            T3[0:64, t3off:t3off + 508]), rsem, eng)
                        push(lambda DE=DE, t3off=t3off, anbase=anbase: DE.dma_start(
                            A[1][0:64, anbase + 508:anbase + 1016],
                            T3[64:128, t3off:t3off + 508]), rsem, eng)
                        push(lambda DE=DE, t3off=t3off, anbase=anbase: DE.dma_start(
                            A[1][0:60, anbase + 1016:anbase + 1270],
                            T3[4:64, t3off:t3off + 254]), rsem, eng)
                        # upper half (+1 row shift) written directly from T3
                        push(lambda DE=DE, t3off=t3off, anbase=anbase: DE.dma_start(
                            A[1][64:128, anbase:anbase + 3 * 127],
                            T3[0:64, t3off + 127:t3off + 508]), rsem, eng)
                        push(lambda DE=DE, t3off=t3off, anbase=anbase: DE.dma_start(
                            A[1][64:128, anbase + 3 * 127:anbase + 7 * 127],
                            T3[64:128, t3off:t3off + 508]), rsem, eng)
                        push(lambda DE=DE, t3off=t3off, anbase=anbase: DE.dma_start(
                            A[1][64:124, anbase + 7 * 127:anbase + 9 * 127],
                            T3[4:64, t3off:t3off + 254]), rsem, eng)
                        t3_free[img] = (rsem, csem[rsem])
                        ready[(img, 1)] = (rsem, csem[rsem])
                    elif l <= 3:
                        soff = (stg_tile % 6) * 2000
                        bo_l = BO[l]
                        push(lambda DE=DE, soff=soff, anbase=anbase, r0=r0, rows=rows,
                             Wn=Wn: DE.dma_start(
                            A[ln][0:64, anbase + r0 * Wn:anbase + (r0 + rows) * Wn],
                            STG[0:64, soff:soff + rows * Wn]), rsem, eng)
                        push(lambda DE=DE, soff=soff, anbase=anbase, r0=r0, rows=rows,
                             Wn=Wn, bo_l=bo_l: DE.dma_start(
                            A[ln][0:64, anbase + (bo_l + r0) * Wn:
                                  anbase + (bo_l + r0 + rows) * Wn],
                            STG[64:128, soff:soff + rows * Wn]), rsem, eng)
                        # upper half (+1 row shift) written directly from STG
                        if t == 0:
                            push(lambda DE=DE, soff=soff, anbase=anbase, rows=rows,
                                 Wn=Wn: DE.dma_start(
                                A[ln][64:128, anbase:anbase + (rows - 1) * Wn],
                                STG[0:64, soff + Wn:soff + rows * Wn]), rsem, eng)
                        else:
                            push(lambda DE=DE, soff=soff, anbase=anbase, r0=r0, rows=rows,
                                 Wn=Wn: DE.dma_start(
                                A[ln][64:128, anbase + (r0 - 1) * Wn:
                                      anbase + (r0 - 1 + rows) * Wn],
                                STG[0:64, soff:soff + rows * Wn]), rsem, eng)
                        push(lambda DE=DE, soff=soff, anbase=anbase, r0=r0, rows=rows,
                             Wn=Wn, bo_l=bo_l: DE.dma_start(
                            A[ln][64:128, anbase + (bo_l + r0 - 1) * Wn:
                                  anbase + (bo_l + r0 - 1 + rows) * Wn],
                            STG[64:128, soff:soff + rows * Wn]), rsem, eng)
                        if t == 0:
                            nsb = S_[ln] - 1
                            push(lambda DE=DE, soff=soff, anbase=anbase, Wn=Wn,
                                 cout=cout, nsb=nsb, bo_l=bo_l: DE.dma_start(
                                A[ln][0:nsb * cout,
                                      anbase + 2 * bo_l * Wn:anbase + (2 * bo_l + 2) * Wn],
                                STG[cout:(nsb + 1) * cout, soff:soff + 2 * Wn]), rsem, eng)
                            push(lambda DE=DE, soff=soff, anbase=anbase, Wn=Wn,
                                 cout=cout, nsb=nsb, bo_l=bo_l: DE.dma_start(
                                A[ln][64:64 + nsb * cout,
                                      anbase + (2 * bo_l - 1) * Wn:
                                      anbase + (2 * bo_l + 1) * Wn],
                                STG[cout:(nsb + 1) * cout, soff:soff + 2 * Wn]), rsem, eng)
                        if t == len(job_tiles) - 1:
                            ready[(img, ln)] = (rsem, csem[rsem])
                    elif l == 4:
                        soff = (stg_tile % 6) * 2000
                        push(lambda DE=DE, soff=soff, anbase=anbase, r0=r0, rows=rows,
                             Wn=Wn: DE.dma_start(
                            A[5][0:128, anbase + r0 * Wn:anbase + (r0 + rows) * Wn],
                            STG[0:128, soff:soff + rows * Wn]), rsem, eng)
                        if t == 0:
                            push(lambda DE=DE, soff=soff, anbase=anbase, Wn=Wn:
                                 DE.dma_start(
                                A[5][0:64, anbase + 64 * Wn:anbase + 66 * Wn],
                                STG[64:128, soff:soff + 2 * Wn]), rsem, eng)
                        stg_cnt = csem[rsem]
                        if t == len(job_tiles) - 1:
                            wait(eng, rsem, csem[rsem])
                            push(lambda DE=DE, anbase=anbase, Lpn=Lpn, Wn=Wn:
                                 DE.dma_start(
                                A[5][0:128, anbase + Lpn:anbase + Lpn + 66 * Wn - 1],
                                A[5][0:128, anbase + 1:anbase + 66 * Wn]), rsem, eng)
                            ready[(img, 5)] = (rsem, csem[rsem])
                    else:  # l == 5 -> A6 (1:2 split)
                        soff = (stg_tile % 6) * 2000
                        for s in range(2):
                            g0 = 64 * s + r0
                            nrows = min(rows, 117 - g0)
                            if nrows <= 0:
                                continue
                            sb = g0 // 32
                            ib = g0 % 32
                            push(lambda DE=DE, soff=soff, anbase=anbase, s=s, sb=sb,
                                 ib=ib, nrows=nrows, Wn=Wn, Wo=Wo: DE.dma_start(
                                A[6][sb * 32:(sb + 1) * 32,
                                     anbase + ib * Wn:anbase + (ib + nrows) * Wn],
                                STG[s * 32:(s + 1) * 32, soff:soff + nrows * Wo]), rsem, eng)
                        if r0 == 32:
                            push(lambda DE=DE, soff=soff, anbase=anbase, Wn=Wn, Wo=Wo:
                                 DE.dma_start(
                                A[6][0:32, anbase + 32 * Wn:anbase + 34 * Wn],
                                STG[0:32, soff:soff + 2 * Wo]), rsem, eng)
                            push(lambda DE=DE, soff=soff, anbase=anbase, Wn=Wn, Wo=Wo:
                                 DE.dma_start(
                                A[6][64:96, anbase + 32 * Wn:anbase + 34 * Wn],
                                STG[32:64, soff:soff + 2 * Wo]), rsem, eng)
                        if r0 == 0:
                            push(lambda DE=DE, soff=soff, anbase=anbase, Wn=Wn, Wo=Wo:
                                 DE.dma_start(
                                A[6][32:64, anbase + 32 * Wn:anbase + 34 * Wn],
                                STG[32:64, soff:soff + 2 * Wo]), rsem, eng)
                        stg_cnt = csem[rsem]
                        if t == len(job_tiles) - 1:
                            wait(eng, rsem, csem[rsem])
                            push(lambda DE=DE, anbase=anbase, Lpn=Lpn, Wn=Wn:
                                 DE.dma_start(
                                A[6][0:128, anbase + Lpn:anbase + Lpn + 35 * Wn - 1],
                                A[6][0:128, anbase + 1:anbase + 35 * Wn]), rsem, eng)
                            ready[(img, 6)] = (rsem, csem[rsem])
                    if l >= 1:
                        if l <= 3:
                            stg_cnt = csem[rsem]
                        stg_free[stg_tile % 6] = (rsem, stg_cnt)
                        stg_tile += 1
                else:
                    # ================= output DMA =================
                    wait('gp', 'sact', my_act)
                    oslot = out_tile % 3
                    ooff = oslot * 16 * 115
                    osem = f'so{oslot}'
                    def mkoa(img=img, r0=r0, ooff=ooff, rows=rows):
                        dv = y[img, 0:1]
                        dv = dataclasses.replace(
                            dv, offset=dv.offset + r0 * 115,
                            ap=[[3680, 3], [13225, 2], [1, rows * 115]])
                        sv = OUTB[0:6, ooff:ooff + rows * 115]
                        return nc.gpsimd.dma_start(dv, sv)
                    push(mkoa, osem)
                    rows3 = min(rows, 115 - 96 - r0)
                    if rows3 > 0:
                        def mkob(img=img, r0=r0, ooff=ooff, rows3=rows3):
                            dv = y[img, 0:1]
                            dv = dataclasses.replace(
                                dv, offset=dv.offset + 3 * 3680 + r0 * 115,
                                ap=[[13225, 2], [1, rows3 * 115]])
                            sv = OUTB[6:8, ooff:ooff + rows3 * 115]
                            return nc.gpsimd.dma_start(dv, sv)
                        push(mkob, osem)
                    outb_free[oslot] = (osem, csem[osem])
                    out_tile += 1
                gtile += 1
            # ---- prefetch next group's inputs during the L4 phase ----
            if l == 4 and (img + 1) % DEPTH == 0:
                for j in range(img + 1, min(img + 1 + DEPTH, NIMG)):
                    emit_input(j)
        return cnt

    with nc.Block() as block:
        @block.tensor
        def _(E):
            walk(E, 'pe')

        @block.scalar
        def _(E):
            walk(E, 'act')

        @block.vector
        def _(E):
            walk(E, 'dve')

        @block.gpsimd
        def _(E):
            walk(E, 'gp')

        @block.sync
        def _(E):
            walk(E, 'sy')

    for cm in reversed(ctxs):
        cm.__exit__(None, None, None)
    return nc


def pack_weights(ws):
    """ws: 7 raw arrays (cout, cin, 3, 3). Returns (w0f fp32, w8 fp8e4)."""
    import ml_dtypes
    sws = [np.sign(w).astype(np.float32) for w in ws]
    # L0: rows k = s*3+ci, cols tap*128 + m, m permuted: m = (s%2)*64 + (s//2)*4 + co
    w0f = np.zeros((96, 9 * 128), np.float32)
    for tap in range(9):
        ki, kj = divmod(tap, 3)
        blk = sws[0][:, :, ki, kj].T  # (cin=3, cout=4)
        for s in range(32):
            sp, j1 = s // 2, s % 2
            m0 = j1 * 64 + sp * 4
            w0f[s * 3:(s + 1) * 3, tap * 128 + m0:tap * 128 + m0 + 4] = blk
    w8 = np.zeros((128, W8_COLS), np.float32)
    for l in range(1, 5):
        cin, cout = CH[l]
        S = S_[l]
        loff = W8OFF[l]
        for r in range(3):
            for s in range(S):
                if l <= 3:
                    m0 = (s % 2) * 64 + (s // 2) * cout
                else:
                    m0 = s * cout
                for half in range(2):
                    blk = sws[l][:, :, half, r].T
                    k0 = half * 64 + s * cin
                    w8[k0:k0 + cin, loff + r * 256 + m0:loff + r * 256 + m0 + cout] = blk
                blk2 = sws[l][:, :, 2, r].T
                w8[s * cin:s * cin + cin,
                   loff + r * 256 + 128 + m0:loff + r * 256 + 128 + m0 + cout] = blk2
    for l in (5, 6):
        cin, cout = CH[l]
        S = S_[l]
        Ml = MW[l]
        loff = W8OFF[l]
        for p, (ta, tb) in enumerate(PAIRS56):
            for s in range(S):
                m0 = s * cout
                blka = sws[l][:, :, ta[0], ta[1]].T
                w8[s * cin:(s + 1) * cin, loff + p * 2 * Ml + m0:
                   loff + p * 2 * Ml + m0 + cout] = blka
                if p < 4:
                    blkb = sws[l][:, :, tb[0], tb[1]].T
                    w8[s * cin:(s + 1) * cin, loff + p * 2 * Ml + Ml + m0:
                       loff + p * 2 * Ml + Ml + m0 + cout] = blkb
    return w0f, w8.astype(ml_dtypes.float8_e4m3)


TRACE = False           # test.py sets these; harness leaves them default
TRACE_DIR = None
LAST_RESULT = None


def kernel(**inputs):
    from concourse.bass_utils import run_bass_kernel_spmd
    inp = np.asarray(inputs['inputs'], np.float32)
    ws = [np.asarray(inputs[f'w{i}']) for i in range(7)]
    w0f, w8 = pack_weights(ws)
    nc = build_program()
    in_maps = []
    for c in range(8):
        in_maps.append({'x': np.ascontiguousarray(inp[c * 8:(c + 1) * 8]),
                        'w0f': w0f, 'w8': w8})
    kw = {}
    if TRACE:
        kw = dict(trace=True, tmpdir=TRACE_DIR)
    res = run_bass_kernel_spmd(nc, in_maps, core_ids=list(range(8)), **kw)
    global LAST_RESULT
    LAST_RESULT = res
    out = np.concatenate([res.results[c]['y'] for c in range(8)], axis=0)
    return out.astype(np.float32)
